# revision 16
# baseline (speedup 1.0000x reference)
"""Trainium2 Bass kernel for nn_AttentionPoolDown (v2).

Math (same reduction as v1 baseline):
  * only the P=128 pool queries matter, attending over L = P + T = 2176 keys;
  * ALiBi bias -sl*|r_q - r_k| decomposes as -sl*(r_q + r_k) + 2*sl*min, with
    min via 32 {0,1} indicator rows; r_k is split hi/lo (hi=2*(r>>1), lo=r&1)
    so every key-side augmented value is EXACT in fp8e4m3;
  * softmax without max-subtraction (|logits| < ~6): p = exp(l), out/den via a
    ones-column appended to V.

v2 changes over the 16.7us baseline:
  * fp8 residual-pair operands: x ~ x8 + r8 (two fp8e4m3 rows).  Scores run as
    DoubleRow-perf-mode fp8 matmuls at 0.5 cycles/row -- the stationary K side
    carries a zero-stride broadcast dim [p, 2, m] so K bytes stay at bf16
    parity while the moving Q side supplies (q8 | qr8) column pairs.  One
    DoubleRow matmul per chunk per source (rope, aug) accumulating in PSUM.
  * the 34 augmented bias rows are loaded ONCE per core (they depend only on
    the batch index) instead of once per head: -0.44MB of DMA per core.
  * exp groups per pair are (8,9) instead of (8,8,1): a 512B-aligned PSUM
    slice never straddles a 2KB bank, so the 9-chunk group is legal; one
    fewer ACT instruction per pair saves its 185ns SBUF-ack overhead.
  * outputs leave through SWDGE kv_writeback descriptors PREPARED EARLY on
    the idle Pool sequencer; after the DVE evacuates an accumulator, a tiny
    Pool read of that tile orders a trigger_dma behind it.  The 625ns HWDGE
    gen + 650ns DGE delay leave the critical path; the tail is
    trigger+transfer+sem (~1.0us shorter).
  * input stream is 9 HWDGE DMAs (lane 0 reused; the 9th carries only its
    lane-FIFO wait, which is legal), ordered so the ACT engine never starves:
    kr/aug first pieces sized for the first 8-chunk exp group.

Scheduling machinery inherited from v1 (walrus allows ONE semaphore wait per
instruction): dep-free dummy-claim matmuls absorb DMA waits, donor nops catch
displaced waits, Tile's drain is split across single-wait nops, and the
const-pool init barrier is skipped.  _repair_multi_waits additionally drops
the spurious Pool->DVE WAR wait that Tile puts on accumulator evacuations
(the writeback PREP only reads tile addresses, never data; the real data read
happens at trigger time, which is ordered behind the evacuation by the Pool
read)."""

import os
import numpy as np
import ml_dtypes

B, H, D, T = 2, 16, 64, 2048
MAX_N, R = 32, 4
P = MAX_N * R           # 128 pool tokens (the queries)
L = P + T               # 2176 keys
THETA = 10000.0
SCALE = 1.0 / np.sqrt(D)
NCHUNK = L // 128       # 17 key chunks
NCORES = 8
PAIRS = (B * H) // NCORES   # 4 (b,h) pairs per core
RA = 33                 # aug rows: 32 indicators + ones
# bias = -sl*(r_q + r_k) + 2*sl*min(r_q, r_k); since r_k = sum_t B_kt the
# -sl*r_k term folds into the indicator rows' q-side coefficients
# (2*sl*B_qt - sl), leaving [B_kt x32, ones] on the key side

WKRQ = 2 * P + L        # 2432 fp8 cols per pair window: [q8|qr8] + 17 chunks
WVA = NCHUNK * 65       # 1105 bf16 cols per pair window in VA
WPT = NCHUNK * P        # 2176 bf16 cols per pair window in PT
# aug tile layout: [QA pairs 0-3 | KA chunks 0-16]
AUG_QA0 = 0
AUG_KA = PAIRS * 2 * P                  # 1024
WAUG = AUG_KA + L                       # 3200

GROUPS = [(0, 8), (8, 9)]

_COMPILED = {}

SKIP_INIT_BARRIER = bool(int(os.environ.get("K_SKIP_INIT_BARRIER", "1")))


def _aug_k_col(c):
    """column offset of aug K chunk c in the AUG tile"""
    return AUG_KA + c * 128


def _rope_pair(x, pos):
    """x: [..., L, 32], pos: [..., L] -> rotary split-half, Dh=32."""
    inv = (1.0 / (THETA ** (np.arange(0, 32, dtype=np.float32)[::2] / 32.0))).astype(np.float32)
    ang = pos[..., :, None] * inv
    c, s = np.cos(ang), np.sin(ang)
    x1, x2 = x[..., :16], x[..., 16:]
    return np.concatenate([x1 * c - x2 * s, x1 * s + x2 * c], axis=-1)


def _resid8(x):
    """fp32 -> (fp8 value, fp8 residual) with value+resid ~ x"""
    h = x.astype(ml_dtypes.float8_e4m3)
    r = (x - h.astype(np.float32)).astype(ml_dtypes.float8_e4m3)
    return h, r


def _host_prep(pool_q, pool_k, pool_v, x_q, x_k, x_v, bias_slopes, regions):
    """Returns (krq [B,H,128,WKRQ] fp8, aug [B,H,RA,2*P] q-side fp8,
    ka [B,RA,L] fp8, va [B,H,128,WVA] bf16)."""
    regions = regions.astype(np.int32)
    n_ids = np.arange(1, MAX_N + 1, dtype=np.int32)

    eq = regions[:, None, :] == n_ids[None, :, None]
    starts = np.argmax(eq, axis=-1).astype(np.float32)
    pool_gpos = (starts[..., None] + 0.5 * np.arange(R, dtype=np.float32)).reshape(B, P)
    gpos = np.concatenate(
        [pool_gpos, np.broadcast_to(np.arange(T, dtype=np.float32), (B, T))], -1)
    pool_ridx = np.broadcast_to(np.repeat(n_ids, R), (B, P))
    ridx = np.concatenate([pool_ridx, regions], -1).astype(np.float32)   # [B,L]

    k = np.concatenate([pool_k, x_k], axis=2)                   # [B,H,L,64]
    gpos_b = gpos[:, None]
    ridx_b = ridx[:, None]
    kr = np.concatenate(
        [_rope_pair(k[..., :32], gpos_b), _rope_pair(k[..., 32:], ridx_b)], -1)
    qr = np.concatenate(
        [_rope_pair(pool_q[..., :32], gpos_b[..., :P]),
         _rope_pair(pool_q[..., 32:], ridx_b[..., :P])], -1)    # [B,H,P,64]
    qr = SCALE * qr

    # --- per-(b,h) KRQ: [128, 2*P + L] fp8 ---
    krT = np.swapaxes(kr, -1, -2)                               # [B,H,64,L]
    qrT = np.swapaxes(qr, -1, -2)                               # [B,H,64,P]
    kh, krs = _resid8(krT)
    qh, qrs = _resid8(qrT)
    krq = np.empty((B, H, 128, WKRQ), ml_dtypes.float8_e4m3)
    # queries first: col block [0,P) = q8, [P,2P) = qr8, duplicated across
    # partition halves (the DoubleRow rhs reads [p, 2, n])
    krq[:, :, :64, :P] = qh
    krq[:, :, 64:, :P] = qh
    krq[:, :, :64, P:2 * P] = qrs
    krq[:, :, 64:, P:2 * P] = qrs
    krq[:, :, :64, 2 * P:] = kh
    krq[:, :, 64:, 2 * P:] = krs

    # --- aug rows ---
    Bind = (ridx[:, None, :] >= n_ids[:, None].astype(np.float32)).astype(np.float32)  # [B,32,L]
    ka = np.empty((B, RA, L), np.float32)
    ka[:, :32] = Bind
    ka[:, 32] = 1.0
    ka8 = ka.astype(ml_dtypes.float8_e4m3)                      # exact

    sl = bias_slopes.astype(np.float32)                         # [H]
    qa = np.empty((B, H, RA, P), np.float32)
    qa[:, :, :32] = sl[None, :, None, None] * (
        2.0 * Bind[:, None, :32, :P] - 1.0)
    qa[:, :, 32] = -sl[None, :, None] * ridx[:, None, :P]
    qah, qars = _resid8(qa)
    aug = np.empty((B, H, RA, 2 * P), ml_dtypes.float8_e4m3)
    aug[..., :P] = qah
    aug[..., P:] = qars

    # --- VA: as v1 ---
    v = np.concatenate([pool_v, x_v], axis=2)                   # [B,H,L,64]
    vaug = np.concatenate([v, np.ones((B, H, L, 1), np.float32)], -1)
    va = vaug.reshape(B, H, NCHUNK, 128, 65).transpose(0, 1, 3, 2, 4).reshape(
        B, H, 128, WVA).astype(ml_dtypes.bfloat16)
    return krq, aug, ka8, va


def _patch_tile_drain():
    """Split Tile's aggregated drain waits across single-wait SP nops (walrus
    rejects >1 semaphore wait per instruction).  Extended over v1: also waits
    on nc._extra_drain_waits (the SWDGE writeback completion sem) and clears
    nc._extra_clear_sems."""
    import bass_rust
    import concourse.tile as tile
    from concourse.vector_clock import ScopedClock
    if getattr(tile.TileContext, "_drain_split_patched", False):
        return

    def patched(self, tick_clock, wait_clock):
        nc = self.nc
        nops = [nc.sync.nop(nofuse=True) for _ in range(20)]
        drain_inst = nc.sync.drain()
        wait_clock.add_sem_waits(
            drain_inst.ins, ScopedClock({None: tick_clock.global_clock}))
        si = drain_inst.ins.sync_info
        waits = list(si.on_wait) if si is not None else []
        # the SWDGE ring sems (DMASW*) bump at the same descriptor
        # completions as the WBDMA sem baked into the writeback descriptors;
        # the WBDMA>=64 wait below subsumes them (and the no-exec cost model
        # never fires them, which would deadlock the drain)
        waits = [w for w in waits
                 if not (w.ant_name or "").startswith("DMASW")]
        for sem, val in getattr(nc, "_extra_drain_waits", []):
            waits.append(bass_rust.SyncWait(
                sync_type="semaphore", id=sem.num, ant_name=sem.name,
                wait_mode="sem-ge-imm", wait_value=val, wait_reg=None))
        prio = getattr(nc, "_drain_wait_prio", {})
        waits.sort(key=lambda w: prio.get(
            (w.ant_name or "").rsplit("_", 1)[0], 50))
        if len(waits) > 1:
            upd = list(si.on_update)
            assert len(waits) - 1 <= len(nops)
            # park waits on the LAST nops: leftover wait-free nops burn
            # their 50ns during idle time instead of after the last sem
            nops = nops[len(nops) - (len(waits) - 1):]
            for nop, w in zip(nops, waits[:-1]):
                old = nop.ins.sync_info
                nupd = list(old.on_update) if old is not None else []
                nop.ins.sync_info = bass_rust.SyncInfo(
                    on_wait=[w], on_update=nupd)
            drain_inst.ins.sync_info = bass_rust.SyncInfo(
                on_wait=[waits[-1]], on_update=upd)
        nc.all_engine_barrier()
        assert self.sems is not None
        popped = nc._tile_sem_poison_stack.pop()
        assert popped is self._sem_poison
        sems = list(self.sems.allocated().values())
        sems.extend(getattr(nc, "_extra_clear_sems", []))
        nc.clear_and_free_semaphores(sems)

    tile.TileContext._drain_and_barrier = patched
    tile.TileContext._drain_split_patched = True


def _patch_skip_init_barrier():
    """Skip Bass.__init__'s const-pool barrier (~1us on the first-DMA path).
    The const pool is memset in the preamble before any engine reads it."""
    import concourse.bass as bass
    if getattr(bass.Bass, "_init_barrier_skip_patched", False):
        return
    orig = bass.Bass.all_engine_barrier

    def patched(self, *, sem_only: bool = False):
        if not getattr(self, "_init_barrier_skipped", False):
            self._init_barrier_skipped = True
            return
        return orig(self, sem_only=sem_only)

    bass.Bass.all_engine_barrier = patched
    bass.Bass._init_barrier_skip_patched = True


def _repair_multi_waits(nc):
    """Walrus rejects >1 semaphore wait per instruction; repair Tile's
    occasional aggregates.  v2 additions: drop waits listed in
    nc._spurious_waits (instruction name -> sem prefix) -- used for the WAR
    edge Tile draws from the writeback PREP's address-only read of the OS
    tile to the DVE evacuation that later writes it."""
    import bass_rust

    DONOR_TYPES = (
        "InstMatmult", "InstNoOp", "InstTensorCopy", "InstActivation",
        "InstMemset", "InstTensorTensor", "InstTensorScalarPtr",
    )
    spurious = getattr(nc, "_spurious_waits", {})
    f = nc.m.functions[0]
    moved = []
    for blk in f.blocks:
        insts = list(blk.instructions)
        donors = {}
        for i in insts:
            eng = i.engine.name
            si = getattr(i, "sync_info", None)
            ws = list(si.on_wait) if si is not None else []
            spfx = spurious.get(i.name)
            if spfx is not None and ws:
                dropped = [w for w in ws
                           if (w.ant_name or "").startswith(spfx)]
                if dropped:
                    ws = [w for w in ws if w not in dropped]
                    upd = list(si.on_update)
                    i.sync_info = bass_rust.SyncInfo(on_wait=ws, on_update=upd)
                    for w in dropped:
                        moved.append((i.name, "<dropped>", w.ant_name,
                                      w.wait_value, "spurious"))
            if len(ws) <= 1:
                if (not ws) and type(i).__name__ in DONOR_TYPES:
                    donors.setdefault(eng, []).append(i)
                continue
            dl = donors.setdefault(eng, [])

            def is_dma(w):
                return (w.ant_name or "").startswith(("DMAHW", "DMASW"))

            def is_self(w):
                return (w.ant_name or "").startswith(eng + "_")

            if eng in ("PE", "DVE"):
                dropped = [w for w in ws if is_self(w)]
                ws = [w for w in ws if not is_self(w)]
                for w in dropped:
                    moved.append((i.name, "<dropped>", w.ant_name,
                                  w.wait_value, "self"))
            engws = [w for w in ws if not is_dma(w)]
            dmaws = [w for w in ws if is_dma(w)]
            while len(engws) + len(dmaws) > 1 and dmaws and dl:
                d = dl.pop()
                dsi = d.sync_info
                dupd = list(dsi.on_update) if dsi is not None else []
                w = dmaws.pop()
                d.sync_info = bass_rust.SyncInfo(on_wait=[w], on_update=dupd)
                moved.append((i.name, d.name, w.ant_name, w.wait_value, "dma"))
            keep = engws + dmaws
            assert len(keep) <= 1, (
                f"_repair_multi_waits: {i.name} ({eng}) still needs "
                f"{[str(w) for w in keep]}; engine-wait moves are unsafe "
                f"(they can cycle on in-order sequencers) -- restructure")
            upd = list(si.on_update)
            i.sync_info = bass_rust.SyncInfo(on_wait=keep, on_update=upd)
    return moved


def _finalize_isa(nc):
    """Raw Bass skips Bacc's library-load insertion and extended-inst
    codegen; without them walrus sees empty .instr bytes ("ISA wrong
    length").  Run both after scheduling."""
    import bass_rust
    from concourse import mybir
    from concourse.library_config import all_libraries, standard
    mask = {}
    for lib in all_libraries:
        for t in lib.instructions:
            mask[t] = mask.get(t, 0) | (1 << lib.index)
    bass_rust.insert_library_loads(nc, mask, len(all_libraries), standard.index)
    # insert_library_loads appends a standard-lib restore (and the block adds
    # a final Pool drain) AFTER the closing barrier; nothing in this launch
    # runs after the barrier, so they only lengthen the sim tail
    blk = nc.m.functions[0].blocks[-1]
    insts = list(blk.instructions)
    keep = len(insts)
    while keep > 0:
        t = type(insts[keep - 1]).__name__
        if t in ("InstDrain", "InstPseudoReloadLibraryIndex") and \
                insts[keep - 1].engine.name == "Pool":
            keep -= 1
        else:
            break
    if keep < len(insts):
        for i in insts[keep:]:
            blk.remove_instruction(i)
    mybir.codegen_inst_isa_subclasses(nc)


def _build_module():
    import concourse.bass as bass
    import concourse.tile as tile
    from concourse import mybir

    _patch_tile_drain()
    if SKIP_INIT_BARRIER:
        _patch_skip_init_barrier()

    f32 = mybir.dt.float32
    bf16 = mybir.dt.bfloat16
    f8 = mybir.dt.float8e4
    i32 = mybir.dt.int32
    DR = mybir.MatmulPerfMode.DoubleRow
    nc = bass.Bass(num_swdge_queues=1)

    # ---- DRAM parameters (per core) -------------------------------------
    krq0a_d = nc.declare_dram_parameter("krq0a", [128, 2 * P + 8 * 128], f8, isOutput=False)
    krq0b_d = nc.declare_dram_parameter("krq0b", [128, 9 * 128], f8, isOutput=False)
    aug_d = nc.declare_dram_parameter("aug", [RA, WAUG], f8, isOutput=False)
    krq1_d = nc.declare_dram_parameter("krq1", [128, WKRQ], f8, isOutput=False)
    krq2_d = nc.declare_dram_parameter("krq2", [128, WKRQ], f8, isOutput=False)
    krq3_d = nc.declare_dram_parameter("krq3", [128, WKRQ], f8, isOutput=False)
    va0_d = nc.declare_dram_parameter("va0", [128, WVA], bf16, isOutput=False)
    va1_d = nc.declare_dram_parameter("va1", [128, WVA], bf16, isOutput=False)
    va23_d = nc.declare_dram_parameter("va23", [128, 2 * WVA], bf16, isOutput=False)
    out_d = nc.declare_dram_parameter("outt", [PAIRS, P, 64], f32, isOutput=True)

    wbsem = nc.alloc_semaphore("WBDMA")
    nc._extra_drain_waits = [(wbsem, 16 * PAIRS)]
    nc._extra_clear_sems = [wbsem]
    nc._spurious_waits = {}

    # drain walk order: early-firing sems first, writeback DMA sem last
    nc._drain_wait_prio = {
        "DMAHW0": 1, "DMAHW1": 2, "DMAHW2": 3, "DMAHW3": 4,
        "DMAHW4": 5, "DMAHW5": 6, "DMAHW6": 7, "DMAHW7": 8,
        "Activation": 21, "PE": 22, "Pool": 23, "DVE": 24,
        "WBDMA": 99,
    }

    with tile.TileContext(nc) as tc:
        with (
            tc.tile_pool(name="krq", bufs=1) as krqpool,
            tc.tile_pool(name="aug", bufs=1) as augpool,
            tc.tile_pool(name="va", bufs=1) as vapool,
            tc.tile_pool(name="pt", bufs=1) as ptpool,
            tc.tile_pool(name="os", bufs=1) as ospool,
            tc.tile_pool(name="epool", bufs=4) as epool,
            tc.tile_pool(name="spsum", bufs=2, space="PSUM") as spsum,
            tc.tile_pool(name="apsum", bufs=1, space="PSUM") as apsum,
            tc.tile_pool(name="dpsum", bufs=1, space="PSUM") as dpsum,
        ):
            # ---- static SBUF tensors ------------------------------------
            KRQ = krqpool.tile([128, PAIRS * WKRQ], f8, name="KRQ")
            AUG = augpool.tile([RA, WAUG], f8, name="AUG")
            VA = vapool.tile([128, PAIRS * WVA], bf16, name="VA")
            PT = ptpool.tile([128, PAIRS * WPT], bf16, name="PT")
            OS = ospool.tile([128, PAIRS * 65], f32, name="OS")
            OSD = ospool.tile([128, PAIRS * 64], f32, name="OSD")
            CTX = ospool.tile([128, 1], i32, name="CTX")

            # ---- dep-free warmups ---------------------------------------
            dumm_all = dpsum.tile([1, 64], f32)
            dumm_ctr = [0]

            def dumm_slot():
                i = dumm_ctr[0]
                dumm_ctr[0] += 1
                return dumm_all[0:1, i:i + 1]

            dumm = dumm_slot()
            CB1 = nc.const_aps.aps[(bf16, 1.0)][0:1, 0:1]
            dwarm0 = epool.tile([1, 1], bf16, tag="dwarm")
            nc.tensor.matmul(dumm, lhsT=CB1, rhs=CB1,
                             start=True, stop=True, skip_group_check=True)
            nc.scalar.copy(dwarm0, CB1)
            # DVE warmups double as the exp zero-bias column and the
            # writeback context-index column
            ZB = epool.tile([128, 1], bf16, tag="zb")
            nc.vector.memset(ZB, 0.0)
            nc.vector.memset(CTX, 0)

            # ---- input DMAs (SP -> HWDGE), urgency order ----------------
            nc.sync.dma_start(out=KRQ[:, :2 * P + 8 * 128], in_=krq0a_d[:])
            nc.sync.dma_start(out=AUG[:], in_=aug_d[:])
            nc.sync.dma_start(out=KRQ[:, 2 * P + 8 * 128:WKRQ], in_=krq0b_d[:])
            nc.sync.dma_start(out=KRQ[:, WKRQ:2 * WKRQ], in_=krq1_d[:])
            nc.sync.dma_start(out=KRQ[:, 2 * WKRQ:3 * WKRQ], in_=krq2_d[:])
            nc.sync.dma_start(out=VA[:, :WVA], in_=va0_d[:])
            nc.sync.dma_start(out=KRQ[:, 3 * WKRQ:], in_=krq3_d[:])
            nc.sync.dma_start(out=VA[:, WVA:2 * WVA], in_=va1_d[:])
            nc.sync.dma_start(out=VA[:, 2 * WVA:], in_=va23_d[:])

            # ---- SWDGE writeback descriptor preps (Pool, early) ---------
            # (_finalize_isa inserts the attn-library load before these)
            # kv_writeback wants in [dhi=128, dho=1, batch=1, ncn=65] and
            # out [batch=1, dhi=128, dho=1, n_ctx=65]; build the singleton
            # strides by hand so its stride-derivation asserts hold.
            from concourse.ap import AP as _AP
            for j in range(PAIRS):
                s = OSD[:, j * 64:(j + 1) * 64]
                in_ap = _AP(s.tensor, s.offset,
                            [list(s.ap[0]), [64, 1], [64, 1], [1, 64]])
                o = out_d[j]
                out_ap = _AP(o.tensor, o.offset,
                             [[P * 64, 1], [64, P], [64, 1], [1, 64]])
                nc.gpsimd.kv_writeback(
                    out_ap, in_ap, CTX[:, 0:1],
                    prepare_only=True, sem=wbsem)

            # ---- ACT claims: zero bias column + exp-table preload -------
            dume = epool.tile([1, 1], f32, tag="dume")
            nc.scalar.copy(dume, ZB[0:1, :])
            dume2 = epool.tile([1, 1], f32, tag="dume2")
            nc.scalar.activation(dume2, ZB[0:1, :],
                                 mybir.ActivationFunctionType.Exp,
                                 bias=ZB[0:1, :])

            state = {}

            def claim(src):
                nc.tensor.matmul(dumm_slot(), lhsT=src, rhs=src,
                                 start=True, stop=True, skip_group_check=True)

            def scores_group(j, gi):
                st = state.setdefault(j, {})
                c0, ng = GROUPS[gi]
                w = j * WKRQ
                qrh = KRQ[:, w:w + 2 * P].rearrange(
                    "p (two n) -> p two n", two=2)
                qa = AUG[:, j * 2 * P:(j + 1) * 2 * P].rearrange(
                    "p (two n) -> p two n", two=2)
                # rope first, aug second: the aug DMA's completion sem fires
                # later than krq0a's; claiming it between per-chunk matmuls
                # would stall the already-ready rope matmuls on the in-order
                # PE stream
                if j == 0 and gi == 0:
                    claim(KRQ[0:1, 0:1])                # krq0a
                elif j == 0 and gi == 1:
                    claim(KRQ[0:1, 2 * P + 8 * 128:2 * P + 8 * 128 + 1])
                elif gi == 0 and j == 1:
                    claim(KRQ[0:1, WKRQ:WKRQ + 1])
                elif gi == 0 and j == 2:
                    claim(KRQ[0:1, 2 * WKRQ:2 * WKRQ + 1])
                elif gi == 0 and j == 3:
                    claim(KRQ[0:1, 3 * WKRQ:3 * WKRQ + 1])
                if j == 0 and gi == 0:
                    claim(AUG[0:1, 0:1])                # aug (whole tile)
                nc.tensor.nop(nofuse=True)
                nc.tensor.nop(nofuse=True)
                sp = spsum.tile([128, 1152], f32, name="sp")
                # rope/aug must stay ADJACENT per chunk: start_tensor_calc
                # marks the whole 2KB PSUM zero-region pending-zero, so a
                # start on slice i+1 before slice i's accumulate would wipe
                # slice i's rope contribution
                for i in range(ng):
                    c = c0 + i
                    kcol = w + 2 * P + c * 128
                    nc.tensor.matmul(
                        sp[:, i * P:(i + 1) * P],
                        lhsT=KRQ[:, kcol:kcol + 128].unsqueeze(1)
                            .broadcast_to([128, 2, 128]),
                        rhs=qrh,
                        start=True, stop=False, perf_mode=DR)
                    acol = _aug_k_col(c)
                    nc.tensor.matmul(
                        sp[:, i * P:(i + 1) * P],
                        lhsT=AUG[:, acol:acol + 128].unsqueeze(1)
                            .broadcast_to([RA, 2, 128]),
                        rhs=qa,
                        start=False, stop=True, perf_mode=DR)
                st.setdefault("sps", []).append(sp)

            def exp_group(j, gi):
                st = state[j]
                c0, ng = GROUPS[gi]
                sp = st["sps"][gi]
                nc.scalar.activation(
                    PT[:, j * WPT + c0 * P:j * WPT + (c0 + ng) * P],
                    sp[:, :ng * P],
                    mybir.ActivationFunctionType.Exp, bias=ZB)

            def pv_group(j, gi):
                st = state[j]
                c0, ng = GROUPS[gi]
                if gi == 0:
                    if j == 0:
                        claim(VA[0:1, 0:1])
                    elif j == 1:
                        claim(VA[0:1, WVA:WVA + 1])
                    elif j == 2:
                        claim(VA[0:1, 2 * WVA:2 * WVA + 1])
                    st["acc"] = apsum.tile([P, 65], f32, name="acc")
                    nc.tensor.nop(nofuse=True)
                    if j >= 1:
                        # absorb the WAR wait on the recycled acc buffer
                        nc.tensor.matmul(st["acc"][0:1, 64:65],
                                         lhsT=CB1, rhs=CB1,
                                         start=True, stop=True,
                                         skip_group_check=True)
                for i in range(ng):
                    c = c0 + i
                    nc.tensor.matmul(
                        st["acc"],
                        lhsT=PT[:, j * WPT + c * P:j * WPT + (c + 1) * P],
                        rhs=VA[:, j * WVA + c * 65:j * WVA + (c + 1) * 65],
                        start=(c == 0), stop=(c == NCHUNK - 1))

            def epilogue(j):
                st = state[j]
                # DVE: out = acc * (1/den), fp32, straight from PSUM
                rc = OS[:, j * 65:j * 65 + 1]
                nc.vector.reciprocal(rc, st["acc"][:, 64:65])
                dv = nc.vector.tensor_scalar(
                    OSD[:, j * 64:(j + 1) * 64], st["acc"][:, 0:64],
                    rc, None, mybir.AluOpType.mult)
                # Tile draws a WAR edge from the PREP's address-only read of
                # OSD to this write; the data read happens at trigger time.
                nc._spurious_waits[dv.ins.name] = ("DMASW", "Pool")
                nc.tensor.matmul(dumm if j == PAIRS - 1 else dumm_slot(),
                                 lhsT=CB1, rhs=CB1,
                                 start=True, stop=True,
                                 skip_group_check=True)

            # software pipeline: scores of pair j+1 interleave with exp/PV of j
            for gi in range(len(GROUPS)):
                scores_group(0, gi)
            for j in range(PAIRS):
                for gi in range(len(GROUPS)):
                    exp_group(j, gi)
                    if j + 1 < PAIRS:
                        scores_group(j + 1, gi)
                    pv_group(j, gi)
                epilogue(j)
            # one trigger fires all 4 prepared writebacks; the phantom write
            # of OSD (signals_writable) orders it behind every recip (WAW)
            # and behind the preps (WAR), entries bind their own src/dst so
            # FIFO order is irrelevant
            trg = nc.gpsimd.trigger_dma(None, signals_writable=(OSD[:],))
            # WAR edges from the preps' phantom OSD reads resolve to DMASW
            # sems that only fire when this trigger itself launches the DMAs
            # -- circular and spurious; the Pool-sem wait (preps + recips
            # committed) is the real ordering.
            nc._spurious_waits[trg.ins.name] = ("DMASW", "Pool")

    _repair_multi_waits(nc)
    _finalize_isa(nc)
    return nc


def _get_nc():
    if "nc" not in _COMPILED:
        _COMPILED["nc"] = _build_module()
    return _COMPILED["nc"]


def kernel(pool_q, pool_k, pool_v, x_q, x_k, x_v, bias_slopes, regions,
           t_mask, n_mask, max_n):
    from concourse.bass_utils import run_bass_kernel_spmd

    krq, aug, ka8, va = _host_prep(
        np.asarray(pool_q, np.float32), np.asarray(pool_k, np.float32),
        np.asarray(pool_v, np.float32), np.asarray(x_q, np.float32),
        np.asarray(x_k, np.float32), np.asarray(x_v, np.float32),
        np.asarray(bias_slopes, np.float32), np.asarray(regions))

    f8np = ml_dtypes.float8_e4m3
    in_maps = []
    for c in range(NCORES):
        b, h0 = c // 4, 4 * (c % 4)
        # aug tile: [QA pairs 0-3 | KA chunks 0-16]
        augt = np.empty((RA, WAUG), f8np)
        for jj in range(PAIRS):
            augt[:, jj * 2 * P:(jj + 1) * 2 * P] = aug[b, h0 + jj]
        augt[:, AUG_KA:] = ka8[b]
        m = {
            "krq0a": np.ascontiguousarray(krq[b, h0, :, :2 * P + 8 * 128]),
            "krq0b": np.ascontiguousarray(krq[b, h0, :, 2 * P + 8 * 128:]),
            "aug": augt,
            "krq1": np.ascontiguousarray(krq[b, h0 + 1]),
            "krq2": np.ascontiguousarray(krq[b, h0 + 2]),
            "krq3": np.ascontiguousarray(krq[b, h0 + 3]),
            "va0": np.ascontiguousarray(va[b, h0]),
            "va1": np.ascontiguousarray(va[b, h0 + 1]),
            "va23": np.ascontiguousarray(
                np.swapaxes(va[b, h0 + 2:h0 + 4], 0, 1).reshape(128, 2 * WVA)),
        }
        in_maps.append(m)

    nc = _get_nc()
    res = run_bass_kernel_spmd(
        nc, in_maps, core_ids=list(range(NCORES)),
        trace=bool(int(os.environ.get("KERNEL_TRACE", "0"))))
    _COMPILED["last_result"] = res

    out = np.empty((B, H, P, 64), np.float32)
    for c in range(NCORES):
        b, h0 = c // 4, 4 * (c % 4)
        out[b, h0:h0 + PAIRS] = res.results[c]["outt"]     # [PAIRS, P, 64]
    return out
